# revision 55
# baseline (speedup 1.0000x reference)
"""Relative-position attention (BERT relative_key_query) on 8 trn2 NeuronCores.

Sharding: head-parallel. Each core owns 2 of 16 heads (all 4 batches):
 - projections computed per-core with tensor-parallel weight slices,
 - attention (with both relative-position score terms) per (batch, head),
 - partial output projection per core; host sums the 8 partials (+bo).

Score pipeline is computed transposed, ST[r, l]:
 - qk:    ST += KT_tile.T @ QT (f16, head-dim-major)
 - term2: SKDT[r, l] = KD[r, 1023-r+l]  -> per-partition-offset (diagonal) DMA
          (fp8), accumulated into the score PSUM via fp8 identity matmul.
 - term1: SQD[l, r]  = QDrev[l, 1023-l+r] (diagonal DMA, fp8) then PE
          transpose via fp8 matmul-with-identity into the score PSUM.
 - softmax over r (partition dim): exp on ACT, denominators for free by
   augmenting V with a ones-column in the AV matmul; normalize per head.
The diagonal DMA relies on SBUF APs being flat element-space: an AP
[[W-1, 128], [1, L]] shifts the read window back one element per partition.
The banded distance-term matmuls run fully in fp8e4 (q/k cast to fp8; the
distance table is pre-scaled x64 on the host for fp8 dynamic range and
scaled back 1/64 in the PSUM->SBUF band copy).
"""

import numpy as np

B = 4
L = 1024
HIDDEN = 1024
HEADS = 16
HEAD = 64
MAXLEN = 1024
NCORES = 8
HPC = HEADS // NCORES          # heads per core = 2
DH2 = HPC * HEAD               # per-core projection width = 128
NT = L // 128                  # 8 row tiles
BAND = 1152                    # skew band width (1151 rounded up)
E2 = 2048                      # padded distance-table width
ESCALE = 64.0                  # host-side fp8 range scaling for dist table

_CACHE = {}


def _build_nc():
    import concourse.mybir as mybir
    import concourse.tile as tile
    from concourse import bacc
    from concourse.ap import AP

    f32 = mybir.dt.float32
    f16 = mybir.dt.float16
    f8 = mybir.dt.float8e4
    EXP = mybir.ActivationFunctionType.Exp
    IDENT = mybir.ActivationFunctionType.Identity

    nc = bacc.Bacc("TRN2", target_bir_lowering=False, debug=False,
                   num_devices=NCORES)

    qT_d = nc.dram_tensor("qT", (B, HIDDEN, L), f16, kind="ExternalInput")
    kT_d = nc.dram_tensor("kT", (B, HIDDEN, L), f16, kind="ExternalInput")
    vT_d = nc.dram_tensor("vT", (B, HIDDEN, L), f16, kind="ExternalInput")
    wq_d = nc.dram_tensor("wq", (128, 8, DH2), f16, kind="ExternalInput")
    wk_d = nc.dram_tensor("wk", (128, 8, DH2), f16, kind="ExternalInput")
    wv_d = nc.dram_tensor("wv", (128, 8, DH2), f16, kind="ExternalInput")
    wo_d = nc.dram_tensor("wo", (DH2, HIDDEN), f16, kind="ExternalInput")
    bqc_d = nc.dram_tensor("bqc", (DH2, 1), f32, kind="ExternalInput")
    bkc_d = nc.dram_tensor("bkc", (DH2, 1), f32, kind="ExternalInput")
    bvb_d = nc.dram_tensor("bvb", (128, DH2), f32, kind="ExternalInput")
    e8_d = nc.dram_tensor("e8", (128, E2), f8, kind="ExternalInput")
    erev8_d = nc.dram_tensor("erev8", (128, E2), f8, kind="ExternalInput")
    ident8_d = nc.dram_tensor("ident8", (128, 128), f8, kind="ExternalInput")
    out_d = nc.dram_tensor("out", (B, L, HIDDEN), f16, kind="ExternalOutput")

    def diag(ap, width, n):
        # read window shifted back one element per partition:
        # out[p, j] = t[p, 127 - p + j] in the tile's flat element space
        return AP(tensor=ap.tensor, offset=ap.offset + 127,
                  ap=[[width - 1, 128], [1, n]])

    with tile.TileContext(nc) as tc:
        import contextlib
        with contextlib.ExitStack() as ctx:
            const = ctx.enter_context(tc.tile_pool(name="const", bufs=1))
            actp = ctx.enter_context(tc.tile_pool(name="act", bufs=8))
            vactp = ctx.enter_context(tc.tile_pool(name="vact", bufs=4))
            projp = ctx.enter_context(tc.tile_pool(name="proj", bufs=2))
            proj8p = ctx.enter_context(tc.tile_pool(name="proj8", bufs=2))
            vaugp = ctx.enter_context(tc.tile_pool(name="vaug", bufs=2 * NT))
            band8p = ctx.enter_context(tc.tile_pool(name="band8", bufs=3))
            skewp = ctx.enter_context(tc.tile_pool(name="skew", bufs=NT + 2))
            epp = ctx.enter_context(tc.tile_pool(name="ep", bufs=8))
            ctxp = ctx.enter_context(tc.tile_pool(name="ctx", bufs=2))
            ctxrawp = ctx.enter_context(tc.tile_pool(name="ctxraw", bufs=2))
            smallp = ctx.enter_context(tc.tile_pool(name="small", bufs=2))
            recipbp = ctx.enter_context(tc.tile_pool(name="recipb", bufs=2))
            outp = ctx.enter_context(tc.tile_pool(name="outp", bufs=4))
            ps_qdst = ctx.enter_context(tc.tile_pool(name="psqdst", bufs=4, space="PSUM"))
            ps_ctx = ctx.enter_context(tc.tile_pool(name="psctx", bufs=2, space="PSUM"))

            # ---- input prefetch: loads for batch b are issued one batch
            #      ahead so the ~20us of DMA lands before the PE needs it ----
            qk_act_all = {}
            vts_all = {}

            def issue_loads(b):
                # two contiguous 128-row chunks per DMA: halves the sync
                # sequencer issue slots that sit ahead of the interleaved
                # phase-1 diagonal DMAs
                qk_act = {"q": [], "k": []}
                vts = []
                for dsrc, key in ((qT_d, "q"), (kT_d, "k"), (vT_d, "v")):
                    for cp in range(4):
                        if key == "v":
                            a = vactp.tile([128, 2, L], f16, tag="vact",
                                           name="va")
                        else:
                            a = actp.tile([128, 2, L], f16, tag="act",
                                          name="qa")
                        nc.sync.dma_start(
                            a[:],
                            dsrc[b, 256 * cp:256 * cp + 256, :].rearrange(
                                "(j p) l -> p j l", p=128))
                        for j in (0, 1):
                            if key == "v":
                                vts.append((a, j))
                            else:
                                qk_act[key].append((a, j))
                qk_act_all[b] = qk_act
                vts_all[b] = vts

            # ---- constants (weights prepacked host-side, contiguous DMA;
            #      wq first so the first projection can start immediately) ----
            wq_sb = const.tile([128, 8, DH2], f16, tag="wq")
            nc.sync.dma_start(wq_sb[:], wq_d[:])
            bqc_sb = const.tile([DH2, 1], f32, tag="bqc")
            nc.sync.dma_start(bqc_sb[:], bqc_d[:])
            issue_loads(0)
            wv_sb = const.tile([128, 8, DH2], f16, tag="wv")
            nc.sync.dma_start(wv_sb[:], wv_d[:])
            wk_sb = const.tile([128, 8, DH2], f16, tag="wk")
            nc.sync.dma_start(wk_sb[:], wk_d[:])
            wo_sb = const.tile([128, HIDDEN], f16, tag="wo")
            nc.sync.dma_start(wo_sb[:], wo_d[:])
            bkc_sb = const.tile([DH2, 1], f32, tag="bkc")
            nc.sync.dma_start(bkc_sb[:], bkc_d[:])
            bvb_sb = const.tile([128, DH2], f32, tag="bvb")
            nc.sync.dma_start(bvb_sb[:], bvb_d[:])
            e8_sb = const.tile([128, E2], f8, tag="e8")
            nc.sync.dma_start(e8_sb[:], e8_d[:])
            erev8_sb = const.tile([128, E2], f8, tag="erev8")
            nc.sync.dma_start(erev8_sb[:], erev8_d[:])
            ident8_sb = const.tile([128, 128], f8, tag="ident8")
            nc.sync.dma_start(ident8_sb[:], ident8_d[:])

            # copy-engine rotation for PSUM->SBUF drains
            # (GPSIMD cannot access PSUM)
            cengines = [nc.vector, nc.scalar]
            cstate = [0]

            def rot_copy(dst, src, scale=1.0, bias=None):
                eng = cengines[cstate[0] % len(cengines)]
                cstate[0] += 1
                if eng is nc.scalar:
                    if bias is None:
                        eng.mul(dst, src, scale)
                    else:
                        eng.activation(dst, src, IDENT, bias=bias, scale=scale)
                else:
                    if bias is None:
                        eng.tensor_scalar_mul(dst, src, scale)
                    else:
                        eng.tensor_scalar(dst, src, bias, None,
                                          mybir.AluOpType.add)

            pending = []

            def tail_norm(pb, pcps, pctx2, half):
                # normalize one 512-wide half of the context as soon as its
                # AV accumulation completes
                hs = slice(512 * half, 512 * half + 512)
                rbs = []
                for h in range(HPC):
                    den = smallp.tile([1, 512], f32, tag="den")
                    if h == 0:
                        nc.scalar.copy(den[:], pcps[h][64:65, hs])
                    else:
                        nc.vector.tensor_copy(den[:], pcps[h][64:65, hs])
                    nc.vector.reciprocal_approx_fast(out=den[:], in_=den[:])
                    recipb = recipbp.tile([64, 512], f32, tag="recipb")
                    nc.gpsimd.partition_broadcast(recipb[:], den[:])
                    rbs.append(recipb)
                for h in range(HPC):
                    craw = ctxrawp.tile([64, 512], f32, tag="ctxraw")
                    if h == 0:
                        nc.scalar.copy(craw[:], pcps[h][0:64, hs])
                    else:
                        nc.vector.tensor_copy(craw[:], pcps[h][0:64, hs])
                    nc.vector.tensor_mul(pctx2[64 * h:64 * h + 64, hs],
                                         craw[:], rbs[h][:])

            def tail_proj(pb, pctx2, half):
                for t in range(4 * half, 4 * half + 4):
                    ob = outp.tile([128, L], f16, tag="ob")
                    for n in (0, 1):
                        op = ps_qdst.tile([128, 512], f32, tag="qdst")
                        nc.tensor.matmul(op[:], pctx2[:, 128 * t:128 * t + 128],
                                         wo_sb[:, 512 * n:512 * n + 512],
                                         start=True, stop=True)
                        if (t + n) % 2 == 0:
                            nc.vector.tensor_copy(
                                ob[:, 512 * n:512 * n + 512], op[:])
                        else:
                            nc.scalar.copy(
                                ob[:, 512 * n:512 * n + 512], op[:])
                    nc.sync.dma_start(out_d[pb, 128 * t:128 * t + 128, :],
                                      ob[:])

            for b in range(B):
                qk_act = qk_act_all.pop(b)
                vts = vts_all.pop(b)

                # ---- q projection, then v (fills PE while q copies drain),
                #      then k projection ----
                def qk_proj(wsb, bcol, tiles, tg):
                    # zero-padded layouts so every matmul contracts K=128
                    # (K=64 streams at half rate on the PE). q16 [128, 2048]
                    # f16: head h at cols 1024h, dims in rows 64h:64h+64,
                    # other rows zero; k16 packed [128, 1024] (the zero side
                    # of the score contraction comes from q16). dst8: one
                    # [128, L] fp8 tile per head, dims in rows 0:64.
                    if tg == "q":
                        dst16 = projp.tile([128, 2 * L], f16, tag="q16",
                                           name="q16")
                        nc.scalar.memzero(dst16[64:128, 0:L])
                        nc.scalar.memzero(dst16[0:64, L:2 * L])
                    else:
                        dst16 = projp.tile([128, L], f16, tag="k16",
                                           name="k16")
                    dst8h = [proj8p.tile([128, L], f8, tag=f"qk8_{h}",
                                         name=f"{tg}8_{h}")
                             for h in range(HPC)]
                    for h in range(HPC):
                        nc.scalar.memzero(dst8h[h][64:128, :])
                    half = []
                    for n in (0, 1):
                        psn = ps_qdst.tile([128, 512], f32, tag="qdst")
                        half.append(psn)
                    for c in range(8):
                        ct, cj = tiles[c]
                        for n in (0, 1):
                            nc.tensor.matmul(half[n][:], wsb[:, c, :],
                                             ct[:, cj, 512 * n:512 * n + 512],
                                             start=(c == 0), stop=(c == 7))
                    e1 = nc.scalar if tg == "q" else nc.vector
                    e2 = nc.vector if tg == "q" else nc.scalar
                    for n in (0, 1):
                        if tg == "q":
                            for h in range(HPC):
                                sp = slice(64 * h, 64 * h + 64)
                                d16 = dst16[sp, 1024 * h + 512 * n:
                                            1024 * h + 512 * n + 512]
                                if e1 is nc.scalar:
                                    e1.activation(d16, half[n][sp, :], IDENT,
                                                  bias=bcol[sp, :], scale=1.0)
                                else:
                                    e1.tensor_scalar(d16, half[n][sp, :],
                                                     bcol[sp, :], None,
                                                     mybir.AluOpType.add)
                        else:
                            d16 = dst16[:, 512 * n:512 * n + 512]
                            if e1 is nc.scalar:
                                e1.activation(d16, half[n][:], IDENT,
                                              bias=bcol[:], scale=1.0)
                            else:
                                e1.tensor_scalar(d16, half[n][:], bcol[:],
                                                 None, mybir.AluOpType.add)
                        for h in range(HPC):
                            sp = slice(64 * h, 64 * h + 64)
                            d8 = dst8h[h][0:64, 512 * n:512 * n + 512]
                            if e2 is nc.scalar:
                                e2.activation(d8, half[n][sp, :], IDENT,
                                              bias=bcol[sp, :], scale=1.0)
                            else:
                                e2.tensor_scalar(d8, half[n][sp, :],
                                                 bcol[sp, :], None,
                                                 mybir.AluOpType.add)
                    return dst16, dst8h

                QT2, QT8 = qk_proj(wq_sb, bqc_sb, qk_act["q"], "q")
                KT2, KT8 = qk_proj(wk_sb, bkc_sb, qk_act["k"], "k")

                if pending:
                    pb, pcps, pctx2 = pending.pop(0)
                    tail_norm(pb, pcps, pctx2, 1)
                    tail_proj(pb, pctx2, 1)

                # ---- phase 1 (banded dist terms, fp8) interleaved with
                #      phase 2 (scores/softmax/AV): band-set PSUM drains are
                #      copy-bound, so alternate them with copy-free score
                #      blocks to keep the PE streaming ----
                skq = []

                def emit_band(t):
                    base = 896 - 128 * t
                    # both terms and both heads share one band tile and one
                    # 4-plane diagonal DMA: plane 2*term+h at col 1024*(2t+h)
                    b8 = band8p.tile([128, 4 * BAND], f8, tag="band8",
                                     name="b8")
                    for term in (0, 1):
                        stat8 = QT8 if term == 0 else KT8
                        emb8 = erev8_sb if term == 0 else e8_sb
                        for h in range(HPC):
                            pb_ = BAND * (2 * term + h)
                            for o, w in ((0, 512), (512, 512), (1024, 128)):
                                qp = ps_qdst.tile([128, 512], f32, tag="qdst",
                                                  name="qp")
                                nc.tensor.matmul(
                                    qp[:, 0:w],
                                    stat8[h][:, 128 * t:128 * t + 128],
                                    emb8[:, base + o:base + o + w],
                                    start=True, stop=True)
                                rot_copy(b8[:, pb_ + o:pb_ + o + w],
                                         qp[:, 0:w], 1.0 / ESCALE)
                    sk = skewp.tile([128, 4 * L], f8, tag="skq", name="sk")
                    src_ap = AP(tensor=b8[:].tensor,
                                offset=b8[:].offset + 127,
                                ap=[[4 * BAND - 1, 128], [BAND, 4], [1, L]])
                    dst_ap = AP(tensor=sk[:].tensor, offset=sk[:].offset,
                                ap=[[4 * L, 128], [L, 4], [1, L]])
                    nc.sync.dma_start(dst_ap, src_ap)
                    skq.append(sk)

                def emit_p2(s, n, cps, vaugs):
                    sts = []
                    for h in range(HPC):
                        st = ps_qdst.tile([128, 512], f32, tag="qdst", name="st")
                        nc.tensor.matmul(
                            st[:],
                            KT2[:, 128 * s:128 * s + 128],
                            QT2[:, 1024 * h + 512 * n:1024 * h + 512 * n + 512],
                            start=True, stop=False)
                        sts.append(st)
                    for h in range(HPC):
                        st = sts[h]
                        for j in range(4):
                            t = 4 * n + j
                            nc.tensor.matmul(
                                st[:, 128 * j:128 * j + 128],
                                skq[t][:, 1024 * h + 128 * s:
                                        1024 * h + 128 * s + 128],
                                ident8_sb[:],
                                start=False, stop=False)
                        # term2 add: identity x skdt chunk
                        nc.tensor.matmul(
                            st[:], ident8_sb[:],
                            skq[s][:, 2048 + 1024 * h + 512 * n:
                                    2048 + 1024 * h + 512 * n + 512],
                            start=False, stop=True)
                        ep = epp.tile([128, 512], f16, tag="ep", name="ep")
                        nc.scalar.activation(ep[:], st[:], EXP, scale=0.125)
                        nc.tensor.matmul(cps[h][0:65, 512 * n:512 * n + 512],
                                         vaugs[s][:, 65 * h:65 * h + 65], ep[:],
                                         start=(s == 0), stop=(s == NT - 1))

                for t in range(4):
                    emit_band(t)

                # ---- v projection + bias + ones columns, after the first
                #      phase-1 block: by now the v loads have long landed and
                #      the projection-epilogue copies have drained ----
                vaugs = []
                for g in range(NT // 2):
                    vps = []
                    for s2 in (0, 1):
                        vp = ps_qdst.tile([128, 512], f32, tag="qdst",
                                          name="vp")
                        vps.append(vp)
                    for c in range(8):
                        ct, cj = vts[c]
                        for s2 in (0, 1):
                            s = 2 * g + s2
                            nc.tensor.matmul(
                                vps[s2][:, 0:DH2],
                                ct[:, cj, 128 * s:128 * s + 128],
                                wv_sb[:, c, :], start=(c == 0), stop=(c == 7))
                    for s2 in (0, 1):
                        vp = vps[s2]
                        va = vaugp.tile([128, 2 * (HEAD + 1)], f16, tag="vaug",
                                        name="va2")
                        for h in range(HPC):
                            nc.vector.scalar_tensor_tensor(
                                va[:, 65 * h:65 * h + 64],
                                vp[:, 64 * h:64 * h + 64], 1.0,
                                bvb_sb[:, 64 * h:64 * h + 64],
                                mybir.AluOpType.mult, mybir.AluOpType.add)
                            nc.vector.memset(va[:, 65 * h + 64:65 * h + 65], 1.0)
                        vaugs.append(va)
                if b + 1 < B:
                    issue_loads(b + 1)
                ctx2 = ctxp.tile([128, L], f16, tag="ctx2")
                cps = []
                for h in range(HPC):
                    cp = ps_ctx.tile([128, L], f32, tag="ctx")
                    cps.append(cp)
                for n in (0, 1):
                    for s in range(NT):
                        emit_p2(s, n, cps, vaugs)
                        if n == 0 and s < 4:
                            emit_band(4 + s)
                        if n == 1 and s == 2:
                            tail_proj(b, ctx2, 0)
                    if n == 0:
                        tail_norm(b, cps, ctx2, 0)
                pending.append((b, cps, ctx2))

            while pending:
                pb, pcps, pctx2 = pending.pop(0)
                tail_norm(pb, pcps, pctx2, 1)
                tail_proj(pb, pctx2, 1)

    nc.compile()
    return nc


def _get_nc():
    if "nc" not in _CACHE:
        _CACHE["nc"] = _build_nc()
    return _CACHE["nc"]


def _prep_in_maps(query, key, value, Wq, bq, Wk, bk, Wv, bv, Wo, bo, dist_emb):
    import ml_dtypes
    f32, f16 = np.float32, np.float16
    f8 = ml_dtypes.float8_e4m3
    qT = np.ascontiguousarray(np.transpose(np.asarray(query, f32), (0, 2, 1)).astype(f16))
    kT = np.ascontiguousarray(np.transpose(np.asarray(key, f32), (0, 2, 1)).astype(f16))
    vT = np.ascontiguousarray(np.transpose(np.asarray(value, f32), (0, 2, 1)).astype(f16))
    E = np.asarray(dist_emb, f32) * ESCALE

    def mk_table(tbl):
        # tbl [2*MAXLEN-1, HEAD] -> [128, E2] fp8, rows = head dims
        # duplicated for both heads' partition groups
        t8 = np.zeros((128, E2), f32)
        t8[0:64, :2 * MAXLEN - 1] = tbl.T
        return t8.astype(f8)

    e8 = mk_table(E)
    erev8 = mk_table(E[::-1])
    ident8 = np.eye(128, dtype=f32).astype(f8)
    in_maps = []
    for c in range(NCORES):
        sl = slice(DH2 * c, DH2 * (c + 1))

        def pack_w(W):
            # [DH2, HIDDEN] slice -> [HIDDEN, DH2] -> [128, 8, DH2]
            # partition-major chunks so the on-chip DMA is contiguous
            wt = np.asarray(W, f32)[sl, :].T.astype(f16)
            return np.ascontiguousarray(
                wt.reshape(8, 128, DH2).transpose(1, 0, 2))
        in_maps.append({
            "qT": qT, "kT": kT, "vT": vT,
            "wq": pack_w(Wq), "wk": pack_w(Wk), "wv": pack_w(Wv),
            "wo": np.ascontiguousarray(np.asarray(Wo, f32)[:, sl].T.astype(f16)),
            "bqc": np.asarray(bq, f32)[sl].reshape(DH2, 1),
            "bkc": np.asarray(bk, f32)[sl].reshape(DH2, 1),
            "bvb": np.tile(np.asarray(bv, f32)[sl].reshape(1, DH2), (128, 1)),
            "e8": e8, "erev8": erev8, "ident8": ident8,
        })
    return in_maps


def run(inputs, trace=False):
    from concourse.bass_utils import run_bass_kernel_spmd
    nc = _get_nc()
    in_maps = _prep_in_maps(**inputs)
    res = run_bass_kernel_spmd(nc, in_maps, core_ids=list(range(NCORES)),
                               trace=trace)
    out = np.zeros((B, L, HIDDEN), np.float32)
    for r in res.results:
        out += r["out"].astype(np.float32)
    out += np.asarray(inputs["bo"], np.float32)[None, None, :]
    return out, res


def kernel(**inputs):
    out, _ = run(inputs, trace=False)
    return out


# revision 57
# speedup vs baseline: 1.0071x; 1.0071x over previous
"""Relative-position attention (BERT relative_key_query) on 8 trn2 NeuronCores.

Sharding: head-parallel. Each core owns 2 of 16 heads (all 4 batches):
 - projections computed per-core with tensor-parallel weight slices,
 - attention (with both relative-position score terms) per (batch, head),
 - partial output projection per core; host sums the 8 partials (+bo).

Score pipeline is computed transposed, ST[r, l]:
 - qk:    ST += KT_tile.T @ QT (f16, head-dim-major)
 - term2: SKDT[r, l] = KD[r, 1023-r+l]  -> per-partition-offset (diagonal) DMA
          (fp8), accumulated into the score PSUM via fp8 identity matmul.
 - term1: SQD[l, r]  = QDrev[l, 1023-l+r] (diagonal DMA, fp8) then PE
          transpose via fp8 matmul-with-identity into the score PSUM.
 - softmax over r (partition dim): exp on ACT, denominators for free by
   augmenting V with a ones-column in the AV matmul; normalize per head.
The diagonal DMA relies on SBUF APs being flat element-space: an AP
[[W-1, 128], [1, L]] shifts the read window back one element per partition.
The banded distance-term matmuls run fully in fp8e4 (q/k cast to fp8; the
distance table is pre-scaled x64 on the host for fp8 dynamic range and
scaled back 1/64 in the PSUM->SBUF band copy).
"""

import numpy as np

B = 4
L = 1024
HIDDEN = 1024
HEADS = 16
HEAD = 64
MAXLEN = 1024
NCORES = 8
HPC = HEADS // NCORES          # heads per core = 2
DH2 = HPC * HEAD               # per-core projection width = 128
NT = L // 128                  # 8 row tiles
BAND = 1152                    # skew band width (1151 rounded up)
E2 = 2048                      # padded distance-table width
ESCALE = 64.0                  # host-side fp8 range scaling for dist table

_CACHE = {}


def _build_nc():
    import concourse.mybir as mybir
    import concourse.tile as tile
    from concourse import bacc
    from concourse.ap import AP

    f32 = mybir.dt.float32
    f16 = mybir.dt.float16
    f8 = mybir.dt.float8e4
    EXP = mybir.ActivationFunctionType.Exp
    IDENT = mybir.ActivationFunctionType.Identity

    nc = bacc.Bacc("TRN2", target_bir_lowering=False, debug=False,
                   num_devices=NCORES)

    qT_d = nc.dram_tensor("qT", (B, HIDDEN, L), f16, kind="ExternalInput")
    kT_d = nc.dram_tensor("kT", (B, HIDDEN, L), f16, kind="ExternalInput")
    vT_d = nc.dram_tensor("vT", (B, HIDDEN, L), f16, kind="ExternalInput")
    wq_d = nc.dram_tensor("wq", (128, 8, DH2), f16, kind="ExternalInput")
    wk_d = nc.dram_tensor("wk", (128, 8, DH2), f16, kind="ExternalInput")
    wv_d = nc.dram_tensor("wv", (128, 8, DH2), f16, kind="ExternalInput")
    wo_d = nc.dram_tensor("wo", (DH2, HIDDEN), f16, kind="ExternalInput")
    bqc_d = nc.dram_tensor("bqc", (DH2, 1), f32, kind="ExternalInput")
    bkc_d = nc.dram_tensor("bkc", (DH2, 1), f32, kind="ExternalInput")
    bvb_d = nc.dram_tensor("bvb", (128, DH2), f32, kind="ExternalInput")
    e8_d = nc.dram_tensor("e8", (128, E2), f8, kind="ExternalInput")
    erev8_d = nc.dram_tensor("erev8", (128, E2), f8, kind="ExternalInput")
    ident8_d = nc.dram_tensor("ident8", (128, 128), f8, kind="ExternalInput")
    out_d = nc.dram_tensor("out", (B, L, HIDDEN), f16, kind="ExternalOutput")

    def diag(ap, width, n):
        # read window shifted back one element per partition:
        # out[p, j] = t[p, 127 - p + j] in the tile's flat element space
        return AP(tensor=ap.tensor, offset=ap.offset + 127,
                  ap=[[width - 1, 128], [1, n]])

    with tile.TileContext(nc) as tc:
        import contextlib
        with contextlib.ExitStack() as ctx:
            const = ctx.enter_context(tc.tile_pool(name="const", bufs=1))
            actp = ctx.enter_context(tc.tile_pool(name="act", bufs=8))
            vactp = ctx.enter_context(tc.tile_pool(name="vact", bufs=4))
            projp = ctx.enter_context(tc.tile_pool(name="proj", bufs=2))
            proj8p = ctx.enter_context(tc.tile_pool(name="proj8", bufs=2))
            vaugp = ctx.enter_context(tc.tile_pool(name="vaug", bufs=2 * NT))
            band8p = ctx.enter_context(tc.tile_pool(name="band8", bufs=4))
            skewp = ctx.enter_context(tc.tile_pool(name="skew", bufs=NT + 12))
            epp = ctx.enter_context(tc.tile_pool(name="ep", bufs=8))
            ctxp = ctx.enter_context(tc.tile_pool(name="ctx", bufs=2))
            ctxrawp = ctx.enter_context(tc.tile_pool(name="ctxraw", bufs=2))
            smallp = ctx.enter_context(tc.tile_pool(name="small", bufs=2))
            recipbp = ctx.enter_context(tc.tile_pool(name="recipb", bufs=2))
            outp = ctx.enter_context(tc.tile_pool(name="outp", bufs=4))
            ps_qdst = ctx.enter_context(tc.tile_pool(name="psqdst", bufs=4, space="PSUM"))
            ps_ctx = ctx.enter_context(tc.tile_pool(name="psctx", bufs=2, space="PSUM"))

            # ---- input prefetch: loads for batch b are issued one batch
            #      ahead so the ~20us of DMA lands before the PE needs it ----
            qk_act_all = {}
            vts_all = {}

            def issue_loads(b):
                # two contiguous 128-row chunks per DMA: halves the sync
                # sequencer issue slots that sit ahead of the interleaved
                # phase-1 diagonal DMAs
                qk_act = {"q": [], "k": []}
                vts = []
                for dsrc, key in ((qT_d, "q"), (kT_d, "k"), (vT_d, "v")):
                    for cp in range(4):
                        if key == "v":
                            a = vactp.tile([128, 2, L], f16, tag="vact",
                                           name="va")
                        else:
                            a = actp.tile([128, 2, L], f16, tag="act",
                                          name="qa")
                        nc.sync.dma_start(
                            a[:],
                            dsrc[b, 256 * cp:256 * cp + 256, :].rearrange(
                                "(j p) l -> p j l", p=128))
                        for j in (0, 1):
                            if key == "v":
                                vts.append((a, j))
                            else:
                                qk_act[key].append((a, j))
                qk_act_all[b] = qk_act
                vts_all[b] = vts

            # ---- constants (weights prepacked host-side, contiguous DMA;
            #      wq first so the first projection can start immediately) ----
            wq_sb = const.tile([128, 8, DH2], f16, tag="wq")
            nc.sync.dma_start(wq_sb[:], wq_d[:])
            bqc_sb = const.tile([DH2, 1], f32, tag="bqc")
            nc.sync.dma_start(bqc_sb[:], bqc_d[:])
            issue_loads(0)
            wv_sb = const.tile([128, 8, DH2], f16, tag="wv")
            nc.sync.dma_start(wv_sb[:], wv_d[:])
            wk_sb = const.tile([128, 8, DH2], f16, tag="wk")
            nc.sync.dma_start(wk_sb[:], wk_d[:])
            wo_sb = const.tile([128, HIDDEN], f16, tag="wo")
            nc.sync.dma_start(wo_sb[:], wo_d[:])
            bkc_sb = const.tile([DH2, 1], f32, tag="bkc")
            nc.sync.dma_start(bkc_sb[:], bkc_d[:])
            bvb_sb = const.tile([128, DH2], f32, tag="bvb")
            nc.sync.dma_start(bvb_sb[:], bvb_d[:])
            e8_sb = const.tile([128, E2], f8, tag="e8")
            nc.sync.dma_start(e8_sb[:], e8_d[:])
            erev8_sb = const.tile([128, E2], f8, tag="erev8")
            nc.sync.dma_start(erev8_sb[:], erev8_d[:])
            ident8_sb = const.tile([128, 128], f8, tag="ident8")
            nc.sync.dma_start(ident8_sb[:], ident8_d[:])

            # copy-engine rotation for PSUM->SBUF drains
            # (GPSIMD cannot access PSUM)
            cengines = [nc.vector, nc.scalar]
            cstate = [0]

            def rot_copy(dst, src, scale=1.0, bias=None):
                eng = cengines[cstate[0] % len(cengines)]
                cstate[0] += 1
                if eng is nc.scalar:
                    if bias is None:
                        eng.mul(dst, src, scale)
                    else:
                        eng.activation(dst, src, IDENT, bias=bias, scale=scale)
                else:
                    if bias is None:
                        eng.tensor_scalar_mul(dst, src, scale)
                    else:
                        eng.tensor_scalar(dst, src, bias, None,
                                          mybir.AluOpType.add)

            pending = []

            def tail_norm(pb, pcps, pctx2, half):
                # normalize one 512-wide half of the context as soon as its
                # AV accumulation completes
                hs = slice(512 * half, 512 * half + 512)
                rbs = []
                for h in range(HPC):
                    den = smallp.tile([1, 512], f32, tag="den")
                    if h == 0:
                        nc.scalar.copy(den[:], pcps[h][64:65, hs])
                    else:
                        nc.vector.tensor_copy(den[:], pcps[h][64:65, hs])
                    nc.vector.reciprocal_approx_fast(out=den[:], in_=den[:])
                    recipb = recipbp.tile([64, 512], f32, tag="recipb")
                    nc.gpsimd.partition_broadcast(recipb[:], den[:])
                    rbs.append(recipb)
                for h in range(HPC):
                    craw = ctxrawp.tile([64, 512], f32, tag="ctxraw")
                    if h == 0:
                        nc.scalar.copy(craw[:], pcps[h][0:64, hs])
                    else:
                        nc.vector.tensor_copy(craw[:], pcps[h][0:64, hs])
                    nc.vector.tensor_mul(pctx2[64 * h:64 * h + 64, hs],
                                         craw[:], rbs[h][:])

            def tail_proj(pb, pctx2, half):
                for t in range(4 * half, 4 * half + 4):
                    ob = outp.tile([128, L], f16, tag="ob")
                    for n in (0, 1):
                        op = ps_qdst.tile([128, 512], f32, tag="qdst")
                        nc.tensor.matmul(op[:], pctx2[:, 128 * t:128 * t + 128],
                                         wo_sb[:, 512 * n:512 * n + 512],
                                         start=True, stop=True)
                        if (t + n) % 2 == 0:
                            nc.vector.tensor_copy(
                                ob[:, 512 * n:512 * n + 512], op[:])
                        else:
                            nc.scalar.copy(
                                ob[:, 512 * n:512 * n + 512], op[:])
                    nc.sync.dma_start(out_d[pb, 128 * t:128 * t + 128, :],
                                      ob[:])

            for b in range(B):
                qk_act = qk_act_all.pop(b)
                vts = vts_all.pop(b)

                # ---- q projection, then v (fills PE while q copies drain),
                #      then k projection ----
                def qk_proj(wsb, bcol, tiles, tg):
                    # zero-padded layouts so every matmul contracts K=128
                    # (K=64 streams at half rate on the PE). q16 [128, 2048]
                    # f16: head h at cols 1024h, dims in rows 64h:64h+64,
                    # other rows zero; k16 packed [128, 1024] (the zero side
                    # of the score contraction comes from q16). dst8: one
                    # [128, L] fp8 tile per head, dims in rows 0:64.
                    if tg == "q":
                        dst16 = projp.tile([128, 2 * L], f16, tag="q16",
                                           name="q16")
                        nc.scalar.memzero(dst16[64:128, 0:L])
                        nc.scalar.memzero(dst16[0:64, L:2 * L])
                    else:
                        dst16 = projp.tile([128, L], f16, tag="k16",
                                           name="k16")
                    dst8h = [proj8p.tile([128, L], f8, tag=f"qk8_{h}",
                                         name=f"{tg}8_{h}")
                             for h in range(HPC)]
                    for h in range(HPC):
                        nc.scalar.memzero(dst8h[h][64:128, :])
                    half = []
                    for n in (0, 1):
                        psn = ps_qdst.tile([128, 512], f32, tag="qdst")
                        half.append(psn)
                    for c in range(8):
                        ct, cj = tiles[c]
                        for n in (0, 1):
                            nc.tensor.matmul(half[n][:], wsb[:, c, :],
                                             ct[:, cj, 512 * n:512 * n + 512],
                                             start=(c == 0), stop=(c == 7))
                    e1 = nc.scalar if tg == "q" else nc.vector
                    e2 = nc.vector if tg == "q" else nc.scalar
                    for n in (0, 1):
                        if tg == "q":
                            for h in range(HPC):
                                sp = slice(64 * h, 64 * h + 64)
                                d16 = dst16[sp, 1024 * h + 512 * n:
                                            1024 * h + 512 * n + 512]
                                if e1 is nc.scalar:
                                    e1.activation(d16, half[n][sp, :], IDENT,
                                                  bias=bcol[sp, :], scale=1.0)
                                else:
                                    e1.tensor_scalar(d16, half[n][sp, :],
                                                     bcol[sp, :], None,
                                                     mybir.AluOpType.add)
                        else:
                            d16 = dst16[:, 512 * n:512 * n + 512]
                            if e1 is nc.scalar:
                                e1.activation(d16, half[n][:], IDENT,
                                              bias=bcol[:], scale=1.0)
                            else:
                                e1.tensor_scalar(d16, half[n][:], bcol[:],
                                                 None, mybir.AluOpType.add)
                        for h in range(HPC):
                            sp = slice(64 * h, 64 * h + 64)
                            d8 = dst8h[h][0:64, 512 * n:512 * n + 512]
                            if e2 is nc.scalar:
                                e2.activation(d8, half[n][sp, :], IDENT,
                                              bias=bcol[sp, :], scale=1.0)
                            else:
                                e2.tensor_scalar(d8, half[n][sp, :],
                                                 bcol[sp, :], None,
                                                 mybir.AluOpType.add)
                    return dst16, dst8h

                QT2, QT8 = qk_proj(wq_sb, bqc_sb, qk_act["q"], "q")
                KT2, KT8 = qk_proj(wk_sb, bkc_sb, qk_act["k"], "k")

                if pending:
                    pb, pcps, pctx2 = pending.pop(0)
                    tail_norm(pb, pcps, pctx2, 1)
                    tail_proj(pb, pctx2, 1)

                # ---- phase 1 (banded dist terms, fp8) interleaved with
                #      phase 2 (scores/softmax/AV): band-set PSUM drains are
                #      copy-bound, so alternate them with copy-free score
                #      blocks to keep the PE streaming ----
                sqd = []
                skdt = []

                def emit_band(t, terms=(0, 1)):
                    base = 896 - 128 * t
                    for term in terms:
                        stat8 = QT8 if term == 0 else KT8
                        emb8 = erev8_sb if term == 0 else e8_sb
                        lsts = sqd if term == 0 else skdt
                        # both heads share one band tile and one diagonal
                        # DMA (3-dim AP): halves the diag issue slots
                        b8 = band8p.tile([128, 2 * BAND], f8, tag="band8",
                                         name="b8")
                        for h in range(HPC):
                            for o, w in ((0, 512), (512, 512), (1024, 128)):
                                qp = ps_qdst.tile([128, 512], f32, tag="qdst",
                                                  name="qp")
                                nc.tensor.matmul(
                                    qp[:, 0:w],
                                    stat8[h][:, 128 * t:128 * t + 128],
                                    emb8[:, base + o:base + o + w],
                                    start=True, stop=True)
                                rot_copy(b8[:, BAND * h + o:BAND * h + o + w],
                                         qp[:, 0:w], 1.0 / ESCALE)
                        sk = skewp.tile([128, 2 * L], f8,
                                        tag=("sqd" if term == 0 else "skdt"),
                                        name="sk")
                        src_ap = AP(tensor=b8[:].tensor,
                                    offset=b8[:].offset + 127,
                                    ap=[[2 * BAND - 1, 128], [BAND, 2], [1, L]])
                        dst_ap = AP(tensor=sk[:].tensor, offset=sk[:].offset,
                                    ap=[[2 * L, 128], [L, 2], [1, L]])
                        nc.sync.dma_start(dst_ap, src_ap)
                        lsts.append(sk)

                def emit_p2(s, n, cps, vaugs):
                    sts = []
                    for h in range(HPC):
                        st = ps_qdst.tile([128, 512], f32, tag="qdst", name="st")
                        nc.tensor.matmul(
                            st[:],
                            KT2[:, 128 * s:128 * s + 128],
                            QT2[:, 1024 * h + 512 * n:1024 * h + 512 * n + 512],
                            start=True, stop=False)
                        sts.append(st)
                    for h in range(HPC):
                        st = sts[h]
                        for j in range(4):
                            t = 4 * n + j
                            nc.tensor.matmul(
                                st[:, 128 * j:128 * j + 128],
                                sqd[t][:, 1024 * h + 128 * s:
                                        1024 * h + 128 * s + 128],
                                ident8_sb[:],
                                start=False, stop=False)
                        # term2 add: identity x skdt chunk
                        nc.tensor.matmul(
                            st[:], ident8_sb[:],
                            skdt[s][:, 1024 * h + 512 * n:
                                    1024 * h + 512 * n + 512],
                            start=False, stop=True)
                        ep = epp.tile([128, 512], f16, tag="ep", name="ep")
                        nc.scalar.activation(ep[:], st[:], EXP, scale=0.125)
                        nc.tensor.matmul(cps[h][0:65, 512 * n:512 * n + 512],
                                         vaugs[s][:, 65 * h:65 * h + 65], ep[:],
                                         start=(s == 0), stop=(s == NT - 1))

                for t in range(4):
                    emit_band(t)

                # ---- v projection + bias + ones columns, after the first
                #      phase-1 block: by now the v loads have long landed and
                #      the projection-epilogue copies have drained ----
                vaugs = []
                for g in range(NT // 2):
                    vps = []
                    for s2 in (0, 1):
                        vp = ps_qdst.tile([128, 512], f32, tag="qdst",
                                          name="vp")
                        vps.append(vp)
                    for c in range(8):
                        ct, cj = vts[c]
                        for s2 in (0, 1):
                            s = 2 * g + s2
                            nc.tensor.matmul(
                                vps[s2][:, 0:DH2],
                                ct[:, cj, 128 * s:128 * s + 128],
                                wv_sb[:, c, :], start=(c == 0), stop=(c == 7))
                    for s2 in (0, 1):
                        vp = vps[s2]
                        va = vaugp.tile([128, 2 * (HEAD + 1)], f16, tag="vaug",
                                        name="va2")
                        for h in range(HPC):
                            nc.vector.scalar_tensor_tensor(
                                va[:, 65 * h:65 * h + 64],
                                vp[:, 64 * h:64 * h + 64], 1.0,
                                bvb_sb[:, 64 * h:64 * h + 64],
                                mybir.AluOpType.mult, mybir.AluOpType.add)
                            nc.vector.memset(va[:, 65 * h + 64:65 * h + 65], 1.0)
                        vaugs.append(va)
                ctx2 = ctxp.tile([128, L], f16, tag="ctx2")
                cps = []
                for h in range(HPC):
                    cp = ps_ctx.tile([128, L], f32, tag="ctx")
                    cps.append(cp)
                for n in (0, 1):
                    for s in range(NT):
                        emit_p2(s, n, cps, vaugs)
                        if n == 0 and s < 4:
                            emit_band(4 + s)
                        if n == 1 and s == 2:
                            tail_proj(b, ctx2, 0)
                    if n == 0:
                        tail_norm(b, cps, ctx2, 0)
                        if b + 1 < B:
                            issue_loads(b + 1)
                pending.append((b, cps, ctx2))

            while pending:
                pb, pcps, pctx2 = pending.pop(0)
                tail_norm(pb, pcps, pctx2, 1)
                tail_proj(pb, pctx2, 1)

    nc.compile()
    return nc


def _get_nc():
    if "nc" not in _CACHE:
        _CACHE["nc"] = _build_nc()
    return _CACHE["nc"]


def _prep_in_maps(query, key, value, Wq, bq, Wk, bk, Wv, bv, Wo, bo, dist_emb):
    import ml_dtypes
    f32, f16 = np.float32, np.float16
    f8 = ml_dtypes.float8_e4m3
    qT = np.ascontiguousarray(np.transpose(np.asarray(query, f32), (0, 2, 1)).astype(f16))
    kT = np.ascontiguousarray(np.transpose(np.asarray(key, f32), (0, 2, 1)).astype(f16))
    vT = np.ascontiguousarray(np.transpose(np.asarray(value, f32), (0, 2, 1)).astype(f16))
    E = np.asarray(dist_emb, f32) * ESCALE

    def mk_table(tbl):
        # tbl [2*MAXLEN-1, HEAD] -> [128, E2] fp8, rows = head dims
        # duplicated for both heads' partition groups
        t8 = np.zeros((128, E2), f32)
        t8[0:64, :2 * MAXLEN - 1] = tbl.T
        return t8.astype(f8)

    e8 = mk_table(E)
    erev8 = mk_table(E[::-1])
    ident8 = np.eye(128, dtype=f32).astype(f8)
    in_maps = []
    for c in range(NCORES):
        sl = slice(DH2 * c, DH2 * (c + 1))

        def pack_w(W):
            # [DH2, HIDDEN] slice -> [HIDDEN, DH2] -> [128, 8, DH2]
            # partition-major chunks so the on-chip DMA is contiguous
            wt = np.asarray(W, f32)[sl, :].T.astype(f16)
            return np.ascontiguousarray(
                wt.reshape(8, 128, DH2).transpose(1, 0, 2))
        in_maps.append({
            "qT": qT, "kT": kT, "vT": vT,
            "wq": pack_w(Wq), "wk": pack_w(Wk), "wv": pack_w(Wv),
            "wo": np.ascontiguousarray(np.asarray(Wo, f32)[:, sl].T.astype(f16)),
            "bqc": np.asarray(bq, f32)[sl].reshape(DH2, 1),
            "bkc": np.asarray(bk, f32)[sl].reshape(DH2, 1),
            "bvb": np.tile(np.asarray(bv, f32)[sl].reshape(1, DH2), (128, 1)),
            "e8": e8, "erev8": erev8, "ident8": ident8,
        })
    return in_maps


def run(inputs, trace=False):
    from concourse.bass_utils import run_bass_kernel_spmd
    nc = _get_nc()
    in_maps = _prep_in_maps(**inputs)
    res = run_bass_kernel_spmd(nc, in_maps, core_ids=list(range(NCORES)),
                               trace=trace)
    out = np.zeros((B, L, HIDDEN), np.float32)
    for r in res.results:
        out += r["out"].astype(np.float32)
    out += np.asarray(inputs["bo"], np.float32)[None, None, :]
    return out, res


def kernel(**inputs):
    out, _ = run(inputs, trace=False)
    return out
